# revision 15
# baseline (speedup 1.0000x reference)
"""GQA attention kernel for 8 Trainium2 NeuronCores.

Problem: B=2, S=2048, D=1024, 16 Q heads / 4 KV heads (GQA), causal,
y = softmax((x@wq+bq)(x@wk+bk)^T / 8, causal) @ (x@wv+bv) @ wo + bo

Sharding: core c -> (batch b = c//4, kv-group g = c%4). Each core computes
its batch's attention for 4 Q heads (= 1 KV head) and the partial output
projection through wo[g*256:(g+1)*256, :]. Host sums the 4 partials per
batch and adds bo_eff = bo + bv_expanded @ wo (the V bias is folded out
of the device kernel).

Per-core design (matmul operands bf16, fp32 PSUM accumulation), driven by
trace evidence that the Scalar-engine Exp is the attention-phase rate
limiter (~1113 ns per [128,1024] tile) and the PE p-state (HAM) throttles
on micro-idles:
  - Projections/out-proj/AV use N=1024 moving operands (bf16 max), which
    halves matmul+LDWEIGHTS count vs N=512.
  - ACT does exp ONLY during attention. Normalization reads PSUM directly
    (reciprocal + multiply), no intermediate copies; bv folded to host.
  - Causal trimming: diagonal key-tiles compute/exp/accumulate only the
    valid query range; per-element masking only on the [128,128] triangle
    block via one constant [128,2,128] 0/1 mask DMA'd from DRAM.
  - Out-projection pieces and v-transposes are interleaved one-per-kt into
    the NEXT query block's attention stream so the PE backfills its idle
    slots while ACT streams exps (keeps HAM at K=8/8).
  - PSUM: pool 'sd' (2 bufs x 2 banks) rotates scores/proj/yps/vps tiles;
    pool 'acc' (2 bufs x 2 banks) rotates kv-proj and the per-mc AV
    accumulators [65,2,512] (row 64 = softmax denominator via ones column
    in vA).
  - Output stored bf16; host accumulates partials in float64.
"""

import os
import sys
from contextlib import ExitStack

import numpy as np
import ml_dtypes

if "/opt/trn_rl_repo" not in sys.path:
    sys.path.insert(0, "/opt/trn_rl_repo")

import concourse.bass as bass
import concourse.tile as tile
from concourse import bacc, mybir
from concourse.masks import make_identity

B, S, D = 2, 2048, 1024
H, KVH, HD = 16, 4, 64
GQ = H // KVH        # 4 q heads per core
DG = GQ * HD         # 256 q dims per core
P = 128
KC = D // P          # 8 contraction chunks over D
NKT = S // P         # 16 key tiles
NQB = S // 512       # 4 query blocks
N_CORES = 8

DT = mybir.dt.float32
DTB = mybir.dt.bfloat16
AF = mybir.ActivationFunctionType
BF16 = ml_dtypes.bfloat16

_CACHE = {}


def build_nc():
    nc = bacc.Bacc(
        "TRN2",
        target_bir_lowering=False,
        debug=False,
        enable_asserts=False,
        num_devices=N_CORES,
    )
    xtd = nc.dram_tensor("xtd", [D, S], DTB, kind="ExternalInput").ap()
    wqd = nc.dram_tensor("wqd", [D, DG], DTB, kind="ExternalInput").ap()
    wkvd = nc.dram_tensor("wkvd", [D, 2 * HD], DTB, kind="ExternalInput").ap()
    wod = nc.dram_tensor("wod", [DG, D], DTB, kind="ExternalInput").ap()
    bqd = nc.dram_tensor("bqd", [DG, 1], DT, kind="ExternalInput").ap()
    bkd = nc.dram_tensor("bkd", [HD, 1], DT, kind="ExternalInput").ap()
    trid = nc.dram_tensor("trid", [P, 2, P], DTB, kind="ExternalInput").ap()
    out_p = nc.dram_tensor("out_p", [S, D], DTB, kind="ExternalOutput").ap()

    with tile.TileContext(nc) as tc, ExitStack() as ctx:
        consts = ctx.enter_context(tc.tile_pool(name="consts", bufs=1))
        etp = ctx.enter_context(tc.tile_pool(name="etp", bufs=6))
        ysb = ctx.enter_context(tc.tile_pool(name="ysb", bufs=2))
        vtp = ctx.enter_context(tc.tile_pool(name="vtp", bufs=4))
        recp = ctx.enter_context(tc.tile_pool(name="recp", bufs=4))
        rbp = ctx.enter_context(tc.tile_pool(name="rbp", bufs=4))
        # PSUM (8 banks of 2KB/partition):
        #   psS 'sd'  slot [128,1024]f32 = 2 banks x 2 bufs = 4 banks
        #       (proj outputs, score pairs [128,2,512], yps, vps)
        #   psA 'acc' slot [*,1024]f32  = 2 banks x 2 bufs = 4 banks
        #       (kv-proj ps2, AV accumulators per mc)
        psS = ctx.enter_context(tc.tile_pool(name="psS", bufs=2, space="PSUM"))
        psA = ctx.enter_context(tc.tile_pool(name="psA", bufs=2, space="PSUM"))

        xT = [consts.tile([P, S], DTB, tag=f"xT{dc}", name=f"xT{dc}") for dc in range(KC)]
        qp = [consts.tile([P, S], DTB, tag=f"qp{mc}", name=f"qp{mc}") for mc in range(2)]
        kT2 = consts.tile([P, S], DTB, tag="kT2")
        vA = consts.tile([P, NKT, HD + 1], DTB, tag="vA")
        oT = [consts.tile([P, S], DTB, tag=f"oT{c}", name=f"oT{c}") for c in range(2)]

        wq_sb = [consts.tile([P, DG], DTB, tag=f"wq{kc}", name=f"wq{kc}") for kc in range(KC)]
        wkv_sb = [consts.tile([P, 2 * HD], DTB, tag=f"wkv{kc}", name=f"wkv{kc}") for kc in range(KC)]
        wo_sb = [consts.tile([P, D], DTB, tag=f"wo{c}", name=f"wo{c}") for c in range(2)]
        bq_sb = [consts.tile([P, 1], DT, tag=f"bq{mc}", name=f"bq{mc}") for mc in range(2)]
        bk_sb = consts.tile([HD, 1], DT, tag="bk")
        tri = consts.tile([P, 2, P], DTB, tag="tri")
        ident = consts.tile([P, P], DTB, tag="ident")

        # ---- DMA loads. Critical-path order: wkv + xT half0 feed the first
        # kv-proj (interleaved per kc across both HW queues so low kc land
        # first); late loads (xT half1, wo, tri) partly on the gpsimd swdge.
        QS = [nc.sync, nc.scalar]
        nc.sync.dma_start(bk_sb, bkd[:, :])
        for kc in range(KC):
            QS[kc % 2].dma_start(wkv_sb[kc], wkvd[kc * P:(kc + 1) * P, :])
            QS[(kc + 1) % 2].dma_start(
                xT[kc][:, 0:1024], xtd[kc * P:(kc + 1) * P, 0:1024])
        for kc in range(KC):
            QS[kc % 2].dma_start(wq_sb[kc], wqd[kc * P:(kc + 1) * P, :])
        for c in range(2):
            QS[c].dma_start(bq_sb[c], bqd[c * P:(c + 1) * P, :])
        for dc in range(KC):
            [nc.sync, nc.scalar, nc.gpsimd][dc % 3].dma_start(
                xT[dc][:, 1024:2048], xtd[dc * P:(dc + 1) * P, 1024:2048])
        nc.gpsimd.dma_start(tri, trid[:, :, :])
        for c in range(2):
            QS[c].dma_start(wo_sb[c], wod[c * P:(c + 1) * P, :])

        make_identity(nc, ident)
        nc.vector.memset(vA[:, :, HD:HD + 1], 1.0)

        # ---- phase 1: projections, N=512 streams per block ----
        vts = []
        for nb in range(4):
            sl = slice(nb * 512, (nb + 1) * 512)
            ps2 = psA.tile([P, 512], DT, tag="acc", name="ps2")
            for kc in range(KC):
                nc.tensor.matmul(
                    ps2, wkv_sb[kc], xT[kc][:, sl],
                    start=(kc == 0), stop=(kc == KC - 1))
            nc.scalar.activation(kT2[0:HD, sl], ps2[0:HD, :], AF.Identity, bias=bk_sb)
            vt = vtp.tile([HD, 512], DTB, tag="vt", name=f"vt{nb}")
            nc.vector.tensor_copy(vt, ps2[HD:2 * HD, :])
            vts.append(vt)
            nc.vector.tensor_copy(kT2[HD:2 * HD, sl], kT2[0:HD, sl])
            for mc in range(2):
                ps = psS.tile([P, 512], DT, tag="sd", name="ps")
                for kc in range(KC):
                    nc.tensor.matmul(
                        ps, wq_sb[kc][:, mc * P:(mc + 1) * P], xT[kc][:, sl],
                        start=(kc == 0), stop=(kc == KC - 1))
                nc.scalar.activation(
                    qp[mc][:, sl], ps, AF.Identity, bias=bq_sb[mc], scale=0.125)

        def v_transpose(kt):
            vps = psS.tile([P, HD], DTB, tag="sd", name="vps")
            nc.tensor.transpose(
                vps, vts[kt // 4][:, (kt % 4) * P:((kt % 4) + 1) * P],
                ident[0:HD, 0:HD])
            nc.vector.tensor_copy(vA[:, kt, 0:HD], vps)

        ydq = [nc.sync, nc.gpsimd]
        ystate = [0]
        ycur = [None]

        def out_proj(st, nb2):
            yps = psS.tile([P, 512], DT, tag="sd", name="yps")
            for c in range(2):
                nc.tensor.matmul(
                    yps, oT[c][:, st * P:(st + 1) * P],
                    wo_sb[c][:, nb2 * 512:(nb2 + 1) * 512],
                    start=(c == 0), stop=(c == 1))
            if nb2 == 0:
                ycur[0] = ysb.tile([P, D], DTB, tag="y", name="yt")
            yt = ycur[0]
            ysl = slice(nb2 * 512, (nb2 + 1) * 512)
            if st >= 12:
                # tail: ACT is idle after the last exp; keep DVE clear
                nc.scalar.activation(yt[:, ysl], yps, AF.Identity)
            else:
                nc.vector.tensor_copy(yt[:, ysl], yps)
            ydq[ystate[0] % 2].dma_start(
                out_p[st * P:(st + 1) * P, ysl], yt[:, ysl])
            ystate[0] += 1

        # v-transposes for the first query block's (diagonal) key tiles
        for kt in range(4):
            v_transpose(kt)

        # ---- phase 2: attention as a flat (qb, kt) step stream, skewed by
        # one step: scores+exp for step i+1 are emitted before the AVs of
        # step i, so ACT always has a full step of exp work queued while
        # the PE streams AVs + backfill pieces. ----
        inserts = [("v", lambda kt=kt: v_transpose(kt)) for kt in range(4, 8)]
        steps = [(qb, kt) for qb in range(NQB) for kt in range(4 * (qb + 1))]
        ets_q = []  # pending (ets, c0) per emitted-but-not-consumed step

        def scores_exp(qb, kt):
            ksl = slice(kt * P, (kt + 1) * P)
            di = kt - 4 * qb
            c0 = di * P if di >= 0 else 0
            ets = []
            for mc in range(2):
                sps = psS.tile([P, 2, 512], DT, tag="sd", name="sps")
                for hh in range(2):
                    r = slice(hh * HD, (hh + 1) * HD)
                    nc.tensor.matmul(
                        sps[:, hh, c0:512], kT2[r, ksl],
                        qp[mc][r, qb * 512 + c0:(qb + 1) * 512],
                        start=True, stop=True)
                et = etp.tile([P, 2, 512], DTB, tag="et")
                nc.scalar.activation(
                    et[:, :, c0:512], sps[:, :, c0:512], AF.Exp)
                if di >= 0:
                    nc.vector.tensor_mul(
                        et[:, :, c0:c0 + P], et[:, :, c0:c0 + P], tri)
                ets.append(et)
            ets_q.append((ets, c0))

        acc = [None]

        def normalize_mc(qb, mc):
            # oT = acc[0:64] * (1/acc[64]); den pair copied to SBUF first
            # (the custom-DVE reciprocal microcode cannot read PSUM)
            qsl = slice(qb * 512, (qb + 1) * 512)
            den = recp.tile([1, 2, 512], DT, tag="den")
            nc.vector.tensor_copy(den, acc[0][mc][HD:HD + 1, :, :])
            rec = recp.tile([1, 2, 512], DT, tag="rec")
            nc.vector.reciprocal_approx_fast(rec, den)
            for hh in range(2):
                rbs = rbp.tile([HD, 512], DT, tag="rbs")
                nc.gpsimd.partition_broadcast(rbs, rec[:, hh, :])
                nc.vector.tensor_mul(
                    oT[mc][hh * HD:(hh + 1) * HD, qsl],
                    acc[0][mc][0:HD, hh], rbs)

        def emit_avs(qb, kt):
            nkt = 4 * (qb + 1)
            ets, c0 = ets_q.pop(0)
            for mc in range(2):
                for hh in range(2):
                    nc.tensor.matmul(
                        acc[0][mc][:, hh, c0:512], vA[:, kt, :],
                        ets[mc][:, hh, c0:512],
                        start=(kt == 0), stop=(kt == nkt - 1))
                if kt == nkt - 1:
                    normalize_mc(qb, mc)

        def pop_insert(kt):
            # out-proj pieces read oT written by the previous block's
            # normalize; keep them out of the first steps after a boundary
            for j, (kind, fn) in enumerate(inserts):
                if kind == "v" or kt >= 2:
                    inserts.pop(j)
                    fn()
                    return

        scores_exp(*steps[0])
        for i, (qb, kt) in enumerate(steps):
            nkt = 4 * (qb + 1)
            if kt == 0:
                acc[0] = [psA.tile([HD + 1, 2, 512], DT, tag="acc",
                                   name=f"acc{mc}") for mc in range(2)]
            last = kt == nkt - 1
            if last:
                # boundary: drain the AVs (and per-mc normalize) first so
                # the normalize chain starts as early as possible
                emit_avs(qb, kt)
            if i + 1 < len(steps):
                scores_exp(*steps[i + 1])
            pop_insert(kt)
            if not last:
                emit_avs(qb, kt)
            else:
                for kt2 in range(4 * qb + 8, min(4 * qb + 12, NKT)):
                    inserts.append(("v", lambda kt2=kt2: v_transpose(kt2)))
                for st in range(qb * 4, qb * 4 + 4):
                    for nb2 in range(2):
                        inserts.append(
                            ("o", lambda st=st, nb2=nb2: out_proj(st, nb2)))
        while inserts:
            inserts.pop(0)[1]()

    nc.compile()
    return nc


def kernel(x, mask, wq, bq, wk, bk, wv, bv, wo, bo):
    x = np.asarray(x, dtype=np.float32)
    wq = np.asarray(wq, dtype=np.float32)
    wk = np.asarray(wk, dtype=np.float32)
    wv = np.asarray(wv, dtype=np.float32)
    wo = np.asarray(wo, dtype=np.float32)
    bq = np.asarray(bq, dtype=np.float32)
    bk = np.asarray(bk, dtype=np.float32)
    bv = np.asarray(bv, dtype=np.float32)
    bo = np.asarray(bo, dtype=np.float32)

    wqb = wq.astype(BF16)
    wkb = wk.astype(BF16)
    wvb = wv.astype(BF16)
    wob = wo.astype(BF16)
    xtb = np.ascontiguousarray(x.transpose(0, 2, 1)).astype(BF16)  # [B, D, S]

    # causal triangle mask for the [128,128] diagonal sub-block: keep q >= k
    trin = np.triu(np.ones((P, P), dtype=np.float32)).astype(BF16)
    trin = np.ascontiguousarray(np.repeat(trin[:, None, :], 2, axis=1))

    in_maps = []
    for c in range(N_CORES):
        b, g = c // 4, c % 4
        sq = slice(g * DG, (g + 1) * DG)
        sk = slice(g * HD, (g + 1) * HD)
        in_maps.append({
            "xtd": xtb[b],
            "wqd": np.ascontiguousarray(wqb[:, sq]),
            "wkvd": np.ascontiguousarray(
                np.concatenate([wkb[:, sk], wvb[:, sk]], axis=1)),
            "wod": np.ascontiguousarray(wob[sq, :]),
            "bqd": np.ascontiguousarray((bq[sq] * 0.125).reshape(DG, 1)),
            "bkd": np.ascontiguousarray(bk[sk].reshape(HD, 1)),
            "trid": trin,
        })

    results = _run(in_maps)

    # host reduction: sum bf16 partials in float64; fold bv through wo
    bv_exp = np.repeat(bv.reshape(KVH, HD), GQ, axis=0).reshape(D)
    bo_eff = bo.astype(np.float64) + bv_exp.astype(np.float64) @ wo.astype(np.float64)
    out = np.empty((B, S, D), dtype=np.float32)
    for b in range(B):
        acc = results[b * 4 + 0]["out_p"].astype(np.float64)
        for g in range(1, 4):
            acc += results[b * 4 + g]["out_p"].astype(np.float64)
        out[b] = (acc + bo_eff).astype(np.float32)
    return out


def _get_runner():
    """Build (once) a jitted shard_map callable executing the compiled
    kernel on 8 cores. Adapted from concourse.bass2jax.run_bass_via_pjrt,
    minus output-buffer donation so the callable is re-invokable for
    timing."""
    if "runner" in _CACHE:
        return _CACHE["runner"]
    import jax
    from jax.experimental.shard_map import shard_map
    from jax.sharding import Mesh, PartitionSpec
    from concourse import bass2jax
    from concourse.bass2jax import _bass_exec_p, install_neuronx_cc_hook

    install_neuronx_cc_hook()
    nc = build_nc()
    partition_name = (
        nc.partition_id_tensor.name if nc.partition_id_tensor else None
    )

    in_names, out_names, out_avals, zero_outs = [], [], [], []
    for alloc in nc.m.functions[0].allocations:
        if not isinstance(alloc, mybir.MemoryLocationSet):
            continue
        name = alloc.memorylocations[0].name
        if alloc.kind == "ExternalInput":
            if name != partition_name:
                in_names.append(name)
        elif alloc.kind == "ExternalOutput":
            out_names.append(name)
            shape = tuple(alloc.tensor_shape)
            dtype = mybir.dt.np(alloc.dtype)
            out_avals.append(jax.core.ShapedArray(shape, dtype))
            zero_outs.append(np.zeros(shape, dtype))
    n_params = len(in_names)
    all_names = in_names + out_names
    if partition_name is not None:
        all_names = all_names + [partition_name]

    def _body(*args):
        operands = list(args)
        if partition_name is not None:
            operands.append(bass2jax.partition_id_tensor())
        outs = _bass_exec_p.bind(
            *operands,
            out_avals=tuple(out_avals),
            in_names=tuple(all_names),
            out_names=tuple(out_names),
            lowering_input_output_aliases=(),
            sim_require_finite=True,
            sim_require_nnan=True,
            nc=nc,
        )
        return tuple(outs)

    if os.environ.get("KERNEL_SIM"):
        devices = jax.devices("cpu")[:N_CORES]
    else:
        devices = jax.devices()[:N_CORES]
    mesh = Mesh(np.asarray(devices), ("core",))
    n_all = n_params + len(out_names)
    sharded = jax.jit(
        shard_map(
            _body,
            mesh=mesh,
            in_specs=(PartitionSpec("core"),) * n_all,
            out_specs=(PartitionSpec("core"),) * len(out_names),
            check_rep=False,
        ),
        keep_unused=True,
    )
    runner = {
        "sharded": sharded,
        "in_names": in_names,
        "out_names": out_names,
        "out_avals": out_avals,
        "zero_outs": zero_outs,
        "mesh": mesh,
        "nc": nc,
    }
    _CACHE["runner"] = runner
    return runner


def _run(in_maps):
    r = _get_runner()
    concat_in = [
        np.concatenate([np.asarray(in_maps[c][n]) for c in range(N_CORES)], axis=0)
        for n in r["in_names"]
    ]
    concat_zeros = [
        np.zeros((N_CORES * z.shape[0], *z.shape[1:]), z.dtype)
        for z in r["zero_outs"]
    ]
    out_arrs = r["sharded"](*concat_in, *concat_zeros)
    _CACHE["last_args"] = (concat_in, concat_zeros)
    return [
        {
            n: np.asarray(out_arrs[i]).reshape(
                N_CORES, *r["out_avals"][i].shape
            )[c]
            for i, n in enumerate(r["out_names"])
        }
        for c in range(N_CORES)
    ]


def bench(iters=10):
    """Re-execute the last-run kernel with device-resident inputs and
    return per-call wall times (s). Outputs stay on device."""
    import time as _time
    import jax
    from jax.sharding import NamedSharding, PartitionSpec

    r = _CACHE["runner"]
    concat_in, concat_zeros = _CACHE["last_args"]
    sh = NamedSharding(r["mesh"], PartitionSpec("core"))
    dev_args = [jax.device_put(a, sh) for a in (*concat_in, *concat_zeros)]
    for a in dev_args:
        a.block_until_ready()
    times = []
    for _ in range(iters):
        t0 = _time.perf_counter()
        outs = r["sharded"](*dev_args)
        for o in outs:
            o.block_until_ready()
        times.append(_time.perf_counter() - t0)
    return times


def profile_exec_ns(outdir="/tmp/kernel_ntff"):
    """Capture an NTFF profile of one execution on all 8 cores and return
    (max_core_span_ns, per_core_span_ns). The span is the on-device NEFF
    execution time: last instruction end - first instruction start."""
    import ctypes
    import glob
    import json
    import shutil
    import jax
    from jax.sharding import NamedSharding, PartitionSpec

    r = _CACHE["runner"]
    concat_in, concat_zeros = _CACHE["last_args"]
    sh = NamedSharding(r["mesh"], PartitionSpec("core"))
    dev_args = [jax.device_put(a, sh) for a in (*concat_in, *concat_zeros)]
    for a in dev_args:
        a.block_until_ready()
    outs = r["sharded"](*dev_args)  # warm
    for o in outs:
        o.block_until_ready()

    lib = ctypes.CDLL("/opt/axon/libaxon_pjrt.so")
    if not hasattr(lib, "axon_start_nrt_profile"):
        return None, None
    lib.axon_start_nrt_profile.argtypes = [
        ctypes.POINTER(ctypes.c_int64), ctypes.c_size_t]
    lib.axon_start_nrt_profile.restype = ctypes.c_int64
    lib.axon_stop_nrt_profile.argtypes = [ctypes.c_char_p]
    lib.axon_stop_nrt_profile.restype = ctypes.c_int64

    shutil.rmtree(outdir, ignore_errors=True)
    os.makedirs(outdir, exist_ok=True)
    ids = (ctypes.c_int64 * N_CORES)(*range(N_CORES))
    rc = lib.axon_start_nrt_profile(ids, N_CORES)
    if rc != 0:
        return None, None
    outs = r["sharded"](*dev_args)
    for o in outs:
        o.block_until_ready()
    n = lib.axon_stop_nrt_profile(str(outdir).encode())
    if n <= 0:
        return None, None

    import gauge.profiler
    from concourse._compat import FishPath

    profile = gauge.profiler.Profile(
        profile_path=FishPath(outdir),
        kernel_dev_mode=True,
        profile_on_exit=False,
        bass_kernel=_CACHE["runner"]["nc"].m,
        offline_processing=True,
        fname="*_body*",
    )
    profile.convert_ntffs_to_json(tuple(range(N_CORES)))
    spans = {}
    for c in range(N_CORES):
        jp = os.path.join(outdir, f"ntff_{c}.json")
        if not os.path.exists(jp):
            continue
        d = json.load(open(jp))
        insts = d.get("instruction", [])
        if not insts:
            continue
        t0 = min(i["timestamp"] for i in insts)
        t1 = max(i["timestamp"] + i["duration"] for i in insts)
        spans[c] = t1 - t0
    if not spans:
        return None, None
    return max(spans.values()), spans


# revision 20
# speedup vs baseline: 1.0184x; 1.0184x over previous
"""GQA attention kernel for 8 Trainium2 NeuronCores.

Problem: B=2, S=2048, D=1024, 16 Q heads / 4 KV heads (GQA), causal,
y = softmax((x@wq+bq)(x@wk+bk)^T / 8, causal) @ (x@wv+bv) @ wo + bo

Sharding: core c -> (batch b = c//4, kv-group g = c%4). Each core computes
its batch's attention for 4 Q heads (= 1 KV head) and the partial output
projection through wo[g*256:(g+1)*256, :]. Host sums the 4 partials per
batch and adds bo_eff = bo + bv_expanded @ wo (the V bias is folded out
of the device kernel).

Per-core design (matmul operands bf16, fp32 PSUM accumulation), driven by
trace evidence that the Scalar-engine Exp is the attention-phase rate
limiter (~1113 ns per [128,1024] tile) and the PE p-state (HAM) throttles
on micro-idles:
  - Projections/out-proj/AV use N=1024 moving operands (bf16 max), which
    halves matmul+LDWEIGHTS count vs N=512.
  - ACT does exp ONLY during attention. Normalization reads PSUM directly
    (reciprocal + multiply), no intermediate copies; bv folded to host.
  - Causal trimming: diagonal key-tiles compute/exp/accumulate only the
    valid query range; per-element masking only on the [128,128] triangle
    block via one constant [128,2,128] 0/1 mask DMA'd from DRAM.
  - Out-projection pieces and v-transposes are interleaved one-per-kt into
    the NEXT query block's attention stream so the PE backfills its idle
    slots while ACT streams exps (keeps HAM at K=8/8).
  - PSUM: pool 'sd' (2 bufs x 2 banks) rotates scores/proj/yps/vps tiles;
    pool 'acc' (2 bufs x 2 banks) rotates kv-proj and the per-mc AV
    accumulators [65,2,512] (row 64 = softmax denominator via ones column
    in vA).
  - Output stored bf16; host accumulates partials in float64.
"""

import os
import sys
from contextlib import ExitStack

import numpy as np
import ml_dtypes

if "/opt/trn_rl_repo" not in sys.path:
    sys.path.insert(0, "/opt/trn_rl_repo")

import concourse.bass as bass
import concourse.tile as tile
from concourse import bacc, mybir
from concourse.masks import make_identity

B, S, D = 2, 2048, 1024
H, KVH, HD = 16, 4, 64
GQ = H // KVH        # 4 q heads per core
DG = GQ * HD         # 256 q dims per core
P = 128
KC = D // P          # 8 contraction chunks over D
NKT = S // P         # 16 key tiles
NQB = S // 512       # 4 query blocks
N_CORES = 8

DT = mybir.dt.float32
DTB = mybir.dt.bfloat16
AF = mybir.ActivationFunctionType
BF16 = ml_dtypes.bfloat16

_CACHE = {}


def build_nc():
    nc = bacc.Bacc(
        "TRN2",
        target_bir_lowering=False,
        debug=False,
        enable_asserts=False,
        num_devices=N_CORES,
    )
    xtd = nc.dram_tensor("xtd", [D, S], DTB, kind="ExternalInput").ap()
    wqd = nc.dram_tensor("wqd", [D, DG], DTB, kind="ExternalInput").ap()
    wkvd = nc.dram_tensor("wkvd", [D, 2 * HD], DTB, kind="ExternalInput").ap()
    wod = nc.dram_tensor("wod", [DG, D], DTB, kind="ExternalInput").ap()
    bqd = nc.dram_tensor("bqd", [DG, 1], DT, kind="ExternalInput").ap()
    bkd = nc.dram_tensor("bkd", [HD, 1], DT, kind="ExternalInput").ap()
    trid = nc.dram_tensor("trid", [P, 2, P], DTB, kind="ExternalInput").ap()
    out_p = nc.dram_tensor("out_p", [S, D], DTB, kind="ExternalOutput").ap()

    with tile.TileContext(nc) as tc, ExitStack() as ctx:
        consts = ctx.enter_context(tc.tile_pool(name="consts", bufs=1))
        etp = ctx.enter_context(tc.tile_pool(name="etp", bufs=8))
        ysb = ctx.enter_context(tc.tile_pool(name="ysb", bufs=3))
        vtp = ctx.enter_context(tc.tile_pool(name="vtp", bufs=4))
        recp = ctx.enter_context(tc.tile_pool(name="recp", bufs=4))
        rbp = ctx.enter_context(tc.tile_pool(name="rbp", bufs=4))
        # PSUM (8 banks of 2KB/partition):
        #   psS 'sd'  slot [128,1024]f32 = 2 banks x 2 bufs = 4 banks
        #       (proj outputs, score pairs [128,2,512], yps, vps)
        #   psA 'acc' slot [*,1024]f32  = 2 banks x 2 bufs = 4 banks
        #       (kv-proj ps2, AV accumulators per mc)
        psS = ctx.enter_context(tc.tile_pool(name="psS", bufs=2, space="PSUM"))
        psA = ctx.enter_context(tc.tile_pool(name="psA", bufs=2, space="PSUM"))

        xT = [consts.tile([P, S], DTB, tag=f"xT{dc}", name=f"xT{dc}") for dc in range(KC)]
        qp = [consts.tile([P, S], DTB, tag=f"qp{mc}", name=f"qp{mc}") for mc in range(2)]
        kT2 = consts.tile([P, S], DTB, tag="kT2")
        vA = consts.tile([P, NKT, HD + 1], DTB, tag="vA")
        oT = [consts.tile([P, S], DTB, tag=f"oT{c}", name=f"oT{c}") for c in range(2)]

        wq_sb = [consts.tile([P, DG], DTB, tag=f"wq{kc}", name=f"wq{kc}") for kc in range(KC)]
        wkv_sb = [consts.tile([P, 2 * HD], DTB, tag=f"wkv{kc}", name=f"wkv{kc}") for kc in range(KC)]
        wo_sb = [consts.tile([P, D], DTB, tag=f"wo{c}", name=f"wo{c}") for c in range(2)]
        bq_sb = [consts.tile([P, 1], DT, tag=f"bq{mc}", name=f"bq{mc}") for mc in range(2)]
        bk_sb = consts.tile([HD, 1], DT, tag="bk")
        tri = consts.tile([P, 2, P], DTB, tag="tri")
        ident = consts.tile([P, P], DTB, tag="ident")

        # ---- DMA loads. Critical-path order: wkv + xT half0 feed the first
        # kv-proj (interleaved per kc across both HW queues so low kc land
        # first); late loads (xT half1, wo, tri) partly on the gpsimd swdge.
        QS = [nc.sync, nc.scalar]
        Q3 = [nc.sync, nc.scalar, nc.gpsimd]
        nc.sync.dma_start(bk_sb, bkd[:, :])
        for kc in range(KC):
            QS[kc % 2].dma_start(wkv_sb[kc], wkvd[kc * P:(kc + 1) * P, :])
            QS[(kc + 1) % 2].dma_start(
                xT[kc][:, 0:512], xtd[kc * P:(kc + 1) * P, 0:512])
        for kc in range(KC):
            QS[kc % 2].dma_start(
                xT[kc][:, 512:1024], xtd[kc * P:(kc + 1) * P, 512:1024])
            QS[(kc + 1) % 2].dma_start(wq_sb[kc], wqd[kc * P:(kc + 1) * P, :])
        for c in range(2):
            QS[c].dma_start(bq_sb[c], bqd[c * P:(c + 1) * P, :])
        for dc in range(KC):
            Q3[dc % 3].dma_start(
                xT[dc][:, 1024:2048], xtd[dc * P:(dc + 1) * P, 1024:2048])
        nc.gpsimd.dma_start(tri, trid[:, :, :])
        for c in range(2):
            QS[c].dma_start(wo_sb[c], wod[c * P:(c + 1) * P, :])

        make_identity(nc, ident)
        nc.vector.memset(vA[:, :, HD:HD + 1], 1.0)

        # ---- phase 1: projections, N=512 streams per block ----
        vts = []
        for nb in range(4):
            sl = slice(nb * 512, (nb + 1) * 512)
            ps2 = psA.tile([P, 512], DT, tag="acc", name="ps2")
            for kc in range(KC):
                nc.tensor.matmul(
                    ps2, wkv_sb[kc], xT[kc][:, sl],
                    start=(kc == 0), stop=(kc == KC - 1))
            nc.scalar.activation(kT2[0:HD, sl], ps2[0:HD, :], AF.Identity, bias=bk_sb)
            vt = vtp.tile([HD, 512], DTB, tag="vt", name=f"vt{nb}")
            nc.vector.tensor_copy(vt, ps2[HD:2 * HD, :])
            vts.append(vt)
            nc.vector.tensor_copy(kT2[HD:2 * HD, sl], kT2[0:HD, sl])
            for mc in range(2):
                ps = psS.tile([P, 512], DT, tag="sd", name="ps")
                for kc in range(KC):
                    nc.tensor.matmul(
                        ps, wq_sb[kc][:, mc * P:(mc + 1) * P], xT[kc][:, sl],
                        start=(kc == 0), stop=(kc == KC - 1))
                nc.scalar.activation(
                    qp[mc][:, sl], ps, AF.Identity, bias=bq_sb[mc], scale=0.125)

        def v_transpose(kt):
            vps = psS.tile([P, HD], DTB, tag="sd", name="vps")
            nc.tensor.transpose(
                vps, vts[kt // 4][:, (kt % 4) * P:((kt % 4) + 1) * P],
                ident[0:HD, 0:HD])
            nc.vector.tensor_copy(vA[:, kt, 0:HD], vps)

        ydq = [nc.sync, nc.gpsimd]
        ystate = [0]
        ycur = [None]

        def out_proj(st, nb2):
            yps = psS.tile([P, 512], DT, tag="sd", name="yps")
            for c in range(2):
                nc.tensor.matmul(
                    yps, oT[c][:, st * P:(st + 1) * P],
                    wo_sb[c][:, nb2 * 512:(nb2 + 1) * 512],
                    start=(c == 0), stop=(c == 1))
            yt = ysb.tile([P, 512], DTB, tag="y", name="yt")
            if st >= 12:
                # tail: ACT is idle after the last exp; keep DVE clear
                nc.scalar.activation(yt, yps, AF.Identity)
            else:
                nc.vector.tensor_copy(yt, yps)
            ydq[ystate[0] % 2].dma_start(
                out_p[st * P:(st + 1) * P, nb2 * 512:(nb2 + 1) * 512], yt)
            ystate[0] += 1

        # v-transposes for the first query block's (diagonal) key tiles
        for kt in range(4):
            v_transpose(kt)

        # ---- phase 2: attention as a flat (qb, kt) step stream, skewed by
        # one step: scores+exp for step i+1 are emitted before the AVs of
        # step i, so ACT always has a full step of exp work queued while
        # the PE streams AVs + backfill pieces. ----
        inserts = [("v", lambda kt=kt: v_transpose(kt)) for kt in range(4, 8)]
        steps = [(qb, kt) for qb in range(NQB) for kt in range(4 * (qb + 1))]
        ets_q = []  # pending (ets, c0) per emitted-but-not-consumed step

        def scores_exp(qb, kt):
            ksl = slice(kt * P, (kt + 1) * P)
            di = kt - 4 * qb
            c0 = di * P if di >= 0 else 0
            ets = []
            for mc in range(2):
                sps = psS.tile([P, 2, 512], DT, tag="sd", name="sps")
                for hh in range(2):
                    r = slice(hh * HD, (hh + 1) * HD)
                    nc.tensor.matmul(
                        sps[:, hh, c0:512], kT2[r, ksl],
                        qp[mc][r, qb * 512 + c0:(qb + 1) * 512],
                        start=True, stop=True)
                et = etp.tile([P, 2, 512], DTB, tag="et")
                nc.scalar.activation(
                    et[:, :, c0:512], sps[:, :, c0:512], AF.Exp)
                if di >= 0:
                    nc.vector.tensor_mul(
                        et[:, :, c0:c0 + P], et[:, :, c0:c0 + P], tri)
                ets.append(et)
            ets_q.append((ets, c0))

        acc = [None]

        def normalize_mc(qb, mc):
            # oT = acc[0:64] * (1/acc[64]); den pair copied to SBUF first
            # (the custom-DVE reciprocal microcode cannot read PSUM)
            qsl = slice(qb * 512, (qb + 1) * 512)
            den = recp.tile([1, 2, 512], DT, tag="den")
            nc.vector.tensor_copy(den, acc[0][mc][HD:HD + 1, :, :])
            rec = recp.tile([1, 2, 512], DT, tag="rec")
            nc.vector.reciprocal_approx_fast(rec, den)
            for hh in range(2):
                rbs = rbp.tile([HD, 512], DT, tag="rbs")
                nc.gpsimd.partition_broadcast(rbs, rec[:, hh, :])
                nc.vector.tensor_mul(
                    oT[mc][hh * HD:(hh + 1) * HD, qsl],
                    acc[0][mc][0:HD, hh], rbs)

        def emit_avs(qb, kt):
            nkt = 4 * (qb + 1)
            ets, c0 = ets_q.pop(0)
            for mc in range(2):
                for hh in range(2):
                    nc.tensor.matmul(
                        acc[0][mc][:, hh, c0:512], vA[:, kt, :],
                        ets[mc][:, hh, c0:512],
                        start=(kt == 0), stop=(kt == nkt - 1))
                if kt == nkt - 1:
                    normalize_mc(qb, mc)

        def pop_insert(kt):
            # out-proj pieces read oT written by the previous block's
            # normalize; keep them out of the first steps after a boundary
            for j, (kind, fn) in enumerate(inserts):
                if kind == "v" or kt >= 2:
                    inserts.pop(j)
                    fn()
                    return

        emitted = [0]

        def emit_scores_until(tgt):
            while emitted[0] < min(tgt, len(steps)):
                scores_exp(*steps[emitted[0]])
                emitted[0] += 1

        emit_scores_until(1)
        for i, (qb, kt) in enumerate(steps):
            nkt = 4 * (qb + 1)
            if kt == 0:
                acc[0] = [psA.tile([HD + 1, 2, 512], DT, tag="acc",
                                   name=f"acc{mc}") for mc in range(2)]
            last = kt == nkt - 1
            if last:
                # boundary: drain the AVs (and per-mc normalize) first so
                # the normalize chain starts as early as possible, then give
                # ACT a 3-step exp queue to chew on while the next block's
                # AVs wait for the acc banks
                emit_avs(qb, kt)
                emit_scores_until(i + 4)
            else:
                emit_scores_until(i + 2)
            pop_insert(kt)
            if not last:
                emit_avs(qb, kt)
            else:
                for kt2 in range(4 * qb + 8, min(4 * qb + 12, NKT)):
                    inserts.append(("v", lambda kt2=kt2: v_transpose(kt2)))
                for st in range(qb * 4, qb * 4 + 4):
                    for nb2 in range(2):
                        inserts.append(
                            ("o", lambda st=st, nb2=nb2: out_proj(st, nb2)))
        while inserts:
            inserts.pop(0)[1]()

    nc.compile()
    return nc


def kernel(x, mask, wq, bq, wk, bk, wv, bv, wo, bo):
    x = np.asarray(x, dtype=np.float32)
    wq = np.asarray(wq, dtype=np.float32)
    wk = np.asarray(wk, dtype=np.float32)
    wv = np.asarray(wv, dtype=np.float32)
    wo = np.asarray(wo, dtype=np.float32)
    bq = np.asarray(bq, dtype=np.float32)
    bk = np.asarray(bk, dtype=np.float32)
    bv = np.asarray(bv, dtype=np.float32)
    bo = np.asarray(bo, dtype=np.float32)

    wqb = wq.astype(BF16)
    wkb = wk.astype(BF16)
    wvb = wv.astype(BF16)
    wob = wo.astype(BF16)
    xtb = np.ascontiguousarray(x.transpose(0, 2, 1)).astype(BF16)  # [B, D, S]

    # causal triangle mask for the [128,128] diagonal sub-block: keep q >= k
    trin = np.triu(np.ones((P, P), dtype=np.float32)).astype(BF16)
    trin = np.ascontiguousarray(np.repeat(trin[:, None, :], 2, axis=1))

    in_maps = []
    for c in range(N_CORES):
        b, g = c // 4, c % 4
        sq = slice(g * DG, (g + 1) * DG)
        sk = slice(g * HD, (g + 1) * HD)
        in_maps.append({
            "xtd": xtb[b],
            "wqd": np.ascontiguousarray(wqb[:, sq]),
            "wkvd": np.ascontiguousarray(
                np.concatenate([wkb[:, sk], wvb[:, sk]], axis=1)),
            "wod": np.ascontiguousarray(wob[sq, :]),
            "bqd": np.ascontiguousarray((bq[sq] * 0.125).reshape(DG, 1)),
            "bkd": np.ascontiguousarray(bk[sk].reshape(HD, 1)),
            "trid": trin,
        })

    results = _run(in_maps)

    # host reduction: sum bf16 partials in float64; fold bv through wo
    bv_exp = np.repeat(bv.reshape(KVH, HD), GQ, axis=0).reshape(D)
    bo_eff = bo.astype(np.float64) + bv_exp.astype(np.float64) @ wo.astype(np.float64)
    out = np.empty((B, S, D), dtype=np.float32)
    for b in range(B):
        acc = results[b * 4 + 0]["out_p"].astype(np.float64)
        for g in range(1, 4):
            acc += results[b * 4 + g]["out_p"].astype(np.float64)
        out[b] = (acc + bo_eff).astype(np.float32)
    return out


def _get_runner():
    """Build (once) a jitted shard_map callable executing the compiled
    kernel on 8 cores. Adapted from concourse.bass2jax.run_bass_via_pjrt,
    minus output-buffer donation so the callable is re-invokable for
    timing."""
    if "runner" in _CACHE:
        return _CACHE["runner"]
    import jax
    from jax.experimental.shard_map import shard_map
    from jax.sharding import Mesh, PartitionSpec
    from concourse import bass2jax
    from concourse.bass2jax import _bass_exec_p, install_neuronx_cc_hook

    install_neuronx_cc_hook()
    nc = build_nc()
    partition_name = (
        nc.partition_id_tensor.name if nc.partition_id_tensor else None
    )

    in_names, out_names, out_avals, zero_outs = [], [], [], []
    for alloc in nc.m.functions[0].allocations:
        if not isinstance(alloc, mybir.MemoryLocationSet):
            continue
        name = alloc.memorylocations[0].name
        if alloc.kind == "ExternalInput":
            if name != partition_name:
                in_names.append(name)
        elif alloc.kind == "ExternalOutput":
            out_names.append(name)
            shape = tuple(alloc.tensor_shape)
            dtype = mybir.dt.np(alloc.dtype)
            out_avals.append(jax.core.ShapedArray(shape, dtype))
            zero_outs.append(np.zeros(shape, dtype))
    n_params = len(in_names)
    all_names = in_names + out_names
    if partition_name is not None:
        all_names = all_names + [partition_name]

    def _body(*args):
        operands = list(args)
        if partition_name is not None:
            operands.append(bass2jax.partition_id_tensor())
        outs = _bass_exec_p.bind(
            *operands,
            out_avals=tuple(out_avals),
            in_names=tuple(all_names),
            out_names=tuple(out_names),
            lowering_input_output_aliases=(),
            sim_require_finite=True,
            sim_require_nnan=True,
            nc=nc,
        )
        return tuple(outs)

    if os.environ.get("KERNEL_SIM"):
        devices = jax.devices("cpu")[:N_CORES]
    else:
        devices = jax.devices()[:N_CORES]
    mesh = Mesh(np.asarray(devices), ("core",))
    n_all = n_params + len(out_names)
    sharded = jax.jit(
        shard_map(
            _body,
            mesh=mesh,
            in_specs=(PartitionSpec("core"),) * n_all,
            out_specs=(PartitionSpec("core"),) * len(out_names),
            check_rep=False,
        ),
        keep_unused=True,
    )
    runner = {
        "sharded": sharded,
        "in_names": in_names,
        "out_names": out_names,
        "out_avals": out_avals,
        "zero_outs": zero_outs,
        "mesh": mesh,
        "nc": nc,
    }
    _CACHE["runner"] = runner
    return runner


def _run(in_maps):
    r = _get_runner()
    concat_in = [
        np.concatenate([np.asarray(in_maps[c][n]) for c in range(N_CORES)], axis=0)
        for n in r["in_names"]
    ]
    concat_zeros = [
        np.zeros((N_CORES * z.shape[0], *z.shape[1:]), z.dtype)
        for z in r["zero_outs"]
    ]
    out_arrs = r["sharded"](*concat_in, *concat_zeros)
    _CACHE["last_args"] = (concat_in, concat_zeros)
    return [
        {
            n: np.asarray(out_arrs[i]).reshape(
                N_CORES, *r["out_avals"][i].shape
            )[c]
            for i, n in enumerate(r["out_names"])
        }
        for c in range(N_CORES)
    ]


def bench(iters=10):
    """Re-execute the last-run kernel with device-resident inputs and
    return per-call wall times (s). Outputs stay on device."""
    import time as _time
    import jax
    from jax.sharding import NamedSharding, PartitionSpec

    r = _CACHE["runner"]
    concat_in, concat_zeros = _CACHE["last_args"]
    sh = NamedSharding(r["mesh"], PartitionSpec("core"))
    dev_args = [jax.device_put(a, sh) for a in (*concat_in, *concat_zeros)]
    for a in dev_args:
        a.block_until_ready()
    times = []
    for _ in range(iters):
        t0 = _time.perf_counter()
        outs = r["sharded"](*dev_args)
        for o in outs:
            o.block_until_ready()
        times.append(_time.perf_counter() - t0)
    return times


def profile_exec_ns(outdir="/tmp/kernel_ntff"):
    """Capture an NTFF profile of one execution on all 8 cores and return
    (max_core_span_ns, per_core_span_ns). The span is the on-device NEFF
    execution time: last instruction end - first instruction start."""
    import ctypes
    import glob
    import json
    import shutil
    import jax
    from jax.sharding import NamedSharding, PartitionSpec

    r = _CACHE["runner"]
    concat_in, concat_zeros = _CACHE["last_args"]
    sh = NamedSharding(r["mesh"], PartitionSpec("core"))
    dev_args = [jax.device_put(a, sh) for a in (*concat_in, *concat_zeros)]
    for a in dev_args:
        a.block_until_ready()
    outs = r["sharded"](*dev_args)  # warm
    for o in outs:
        o.block_until_ready()

    lib = ctypes.CDLL("/opt/axon/libaxon_pjrt.so")
    if not hasattr(lib, "axon_start_nrt_profile"):
        return None, None
    lib.axon_start_nrt_profile.argtypes = [
        ctypes.POINTER(ctypes.c_int64), ctypes.c_size_t]
    lib.axon_start_nrt_profile.restype = ctypes.c_int64
    lib.axon_stop_nrt_profile.argtypes = [ctypes.c_char_p]
    lib.axon_stop_nrt_profile.restype = ctypes.c_int64

    shutil.rmtree(outdir, ignore_errors=True)
    os.makedirs(outdir, exist_ok=True)
    ids = (ctypes.c_int64 * N_CORES)(*range(N_CORES))
    rc = lib.axon_start_nrt_profile(ids, N_CORES)
    if rc != 0:
        return None, None
    outs = r["sharded"](*dev_args)
    for o in outs:
        o.block_until_ready()
    n = lib.axon_stop_nrt_profile(str(outdir).encode())
    if n <= 0:
        return None, None

    import gauge.profiler
    from concourse._compat import FishPath

    profile = gauge.profiler.Profile(
        profile_path=FishPath(outdir),
        kernel_dev_mode=True,
        profile_on_exit=False,
        bass_kernel=_CACHE["runner"]["nc"].m,
        offline_processing=True,
        fname="*_body*",
    )
    profile.convert_ntffs_to_json(tuple(range(N_CORES)))
    spans = {}
    for c in range(N_CORES):
        jp = os.path.join(outdir, f"ntff_{c}.json")
        if not os.path.exists(jp):
            continue
        d = json.load(open(jp))
        insts = d.get("instruction", [])
        if not insts:
            continue
        t0 = min(i["timestamp"] for i in insts)
        t1 = max(i["timestamp"] + i["duration"] for i in insts)
        spans[c] = t1 - t0
    if not spans:
        return None, None
    return max(spans.values()), spans
